# revision 15
# baseline (speedup 1.0000x reference)
"""Trainium2 Bass kernel for nn_Decoder — data-parallel, zero-collective design.

Each core owns 4 batches end-to-end:
  - 2-layer LSTM scan, weights resident in fp8-e4m3 (scaled x8, compensated
    in ACT scale), gates in [4,1024] PSUM quarters via DoubleRow fp8 matmuls.
  - P@W0[:EMB] folded host-side: the embedding input contributes one padded
    256-DR chunk (K=128 real) instead of two (K=512).
  - ctx contribution algebraically moved to attention weights:
    gates_ctx = p_att @ (enc @ W0c).  encW = enc@W0c is precomputed ON DEVICE
    once (enc is step-invariant), so the per-step ctx K drops 1024 -> 128
    (bf16 block-diag p stationaries; batch pair packed at partition rows
    0-55 / 64-119 so PE transposes can produce them directly via the
    tile_position col-offset).
  - Attention softmax batched over the 4 batches ([4,56] one exp chain).
  - CE over full vocab for own 188 (t,b) rows; ctx part of hproj uses
    encWo = enc@Wo_ctx (also precomputed) with the stored p columns.
    Partial loss written out; host sums the 8 partials. NO collectives.
"""
import sys
sys.path.insert(0, '/opt/trn_rl_repo')

import numpy as np
import ml_dtypes
import concourse.bass as bass
import concourse.bacc as bacc
import concourse.tile as tile
import concourse.mybir as mybir
from concourse import bass_utils
from concourse.masks import make_identity

AF = mybir.ActivationFunctionType
ALU = mybir.AluOpType
AX = mybir.AxisListType
PM = mybir.MatmulPerfMode
F32 = mybir.dt.float32
BF16 = mybir.dt.bfloat16
FP8 = mybir.dt.float8e4
I32 = mybir.dt.int32

NCORES = 8
S, B, C = 56, 32, 1024
T = 48
H = 1024
EMB = 512
RANK = 128
VOCAB = 32000
TS = T - 1          # 47
BS = B // NCORES    # 4
NCH = 8             # 128-chunks of H/C
KB = 4              # 256-blocks of H (DoubleRow K-chunks)
NROW = TS * BS      # 188
PB = 16             # padded batch block in fp8 DR state (16B stride align)
SPAD = 64           # padded per-batch row block in pair stacking
WS = 8.0            # fp8 weight pre-scale
VCHUNKS = [512] * 62 + [256]
assert sum(VCHUNKS) == VOCAB

BF16NP = ml_dtypes.bfloat16
FP8NP = ml_dtypes.float8_e4m3


def build():
    nc = bacc.Bacc("TRN2", target_bir_lowering=False, debug=False,
                   num_devices=NCORES)

    def din(name, shape, dt):
        return nc.dram_tensor(name, shape, dt, kind="ExternalInput").ap()

    # fp8 weights, host layout [128, (kb, dr, nc, 512)]
    w0e = din("w0e", [128, 1 * 2 * 4096], FP8)   # P@W0[:EMB] folded, K padded
    u0 = din("u0", [128, 4 * 2 * 4096], FP8)
    w1 = din("w1", [128, 4 * 2 * 4096], FP8)
    u1 = din("u1", [128, 4 * 2 * 4096], FP8)
    wc = din("wc", [H, 4 * H], BF16)             # W0[EMB:], for encW precompute
    wo = din("wo", [H + C, EMB], BF16)
    boT = din("boT", [128, 4], F32)
    pTb = din("pTb", [EMB, RANK], BF16)
    # enc hidden-major, pair-padded: [128, (ch, pair, 2*SPAD)]
    encTP = din("encTP", [128, NCH * 2 * 2 * SPAD], BF16)
    smaskT = din("smaskT", [BS, 1], F32)
    dmask = din("dmask", [BS, BS * S], F32)
    h0q_i = din("h0q_i", [128, KB * 2 * PB], FP8)
    h1q_i = din("h1q_i", [128, KB * 2 * PB], FP8)
    e_rows = din("e_rows", [VOCAB, RANK], F32)
    eT = din("eT", [RANK, VOCAB], BF16)
    tokidx = din("tokidx", [NROW, 1], I32)
    labidx = din("labidx", [NROW, 1], I32)
    vmask = din("vmask", [128, 2], F32)

    out_loss = nc.dram_tensor("loss", [1, 1], F32, kind="ExternalOutput").ap()

    with tile.TileContext(nc, num_cores=NCORES) as tc:
        with tc.tile_pool(name="consts", bufs=1) as consts, \
             tc.tile_pool(name="wpool", bufs=1) as wpool, \
             tc.tile_pool(name="state", bufs=1) as state:

            id4 = consts.tile([128, 128], F32)
            make_identity(nc, id4)
            ones128 = consts.tile([128, 1], F32)
            nc.gpsimd.memset(ones128[:], 1.0)

            def wtile(ap, shape, dt, tag):
                t_ = wpool.tile(shape, dt, tag=tag)
                nc.sync.dma_start(t_[:], ap[:])
                return t_

            def wtile_ch(ap, nchunk, width, dt, tag):
                t_ = wpool.tile([128, nchunk * width], dt, tag=tag)
                nc.sync.dma_start(
                    t_[:].rearrange("p (c g) -> p c g", c=nchunk),
                    ap[:].rearrange("(c p) g -> p c g", p=128))
                return t_

            w0e_s = wtile(w0e, [128, 1 * 2 * 4096], FP8, tag="w0e_s")
            u0_s = wtile(u0, [128, 4 * 2 * 4096], FP8, tag="u0_s")
            w1_s = wtile(w1, [128, 4 * 2 * 4096], FP8, tag="w1_s")
            u1_s = wtile(u1, [128, 4 * 2 * 4096], FP8, tag="u1_s")
            boT_s = wtile(boT, [128, 4], F32, tag="boT_s")
            pT_s = wtile_ch(pTb, 4, RANK, BF16, tag="pT_s")
            encTP_s = wtile(encTP, [128, NCH * 2 * 2 * SPAD], BF16,
                            tag="encTP_s")
            smaskT_s = wtile(smaskT, [BS, 1], F32, tag="smaskT_s")
            dmask_s = wtile(dmask, [BS, BS * S], F32, tag="dmask_s")
            vmask_s = wtile(vmask, [128, 2], F32, tag="vmask_s")

            def encp(ch, pair):        # [128, 128] pair-padded enc cols
                return encTP_s[:].rearrange("p (c q s) -> p c q s",
                                            c=NCH, q=2)[:, ch, pair, :]

            def encj(ch, j):           # [128, S] single-batch enc cols
                return encTP_s[:].rearrange("p (c q s) -> p c q s",
                                            c=NCH, q=2)[:, ch, j // 2,
                                                        (j % 2) * SPAD:
                                                        (j % 2) * SPAD + S]

            # fp8 DR-layout state [128, (kb, dr, PB)], batch block padded
            h0q = state.tile([128, KB * 2 * PB], FP8)
            nc.sync.dma_start(h0q[:], h0q_i[:])
            h1q = state.tile([128, KB * 2 * PB], FP8)
            nc.sync.dma_start(h1q[:], h1q_i[:])
            c0 = state.tile([BS, H], F32)
            nc.gpsimd.memset(c0[:], 0.0)
            c1 = state.tile([BS, H], F32)
            nc.gpsimd.memset(c1[:], 0.0)

            # embedding stationary, DR layout [128, (dr, t, PB)], dr row 1 zero
            embq = state.tile([128, 2 * TS * PB], FP8)
            nc.gpsimd.memset(embq[:], 0.0)
            featsT = state.tile([128, NCH * NROW], BF16)   # h1 hidden-major
            elab = state.tile([128, 2 * RANK], F32)
            sum_e = state.tile([128, 2], F32)
            nc.gpsimd.memset(sum_e[:], 1.0)
            lab_ll = state.tile([128, 2], F32)
            nc.gpsimd.memset(lab_ll[:], 0.0)

            # stored attention p columns (pair-packed rows), used BOTH as the
            # per-step ctx-gates stationary and the CE ctx moving operand.
            pTce_all = state.tile([128, 2 * (NROW + 4)], BF16)
            nc.gpsimd.memset(pTce_all[:], 0.0)

            def pTce(pair):
                return pTce_all[:].rearrange("p (q c) -> p q c",
                                             q=2)[:, pair, :]

            # enc@W0c (x WS) and enc@Wo_ctx, pair-packed rows
            encW = [state.tile([128, 4 * H], BF16, tag=f"encW{i}",
                               name=f"encW{i}") for i in range(2)]
            encWo = [state.tile([128, EMB], BF16, tag=f"encWo{i}",
                                name=f"encWo{i}") for i in range(2)]

            def wslice(w, nkb, kb, q, half):
                # [128, (kb dr n)] -> [128, 2, 512] for N-chunk 2q+half
                return w[:].rearrange("p (kb dr n) -> p kb dr n",
                                      kb=nkb, dr=2)[:, kb, :,
                                                    (2 * q + half) * 512:
                                                    (2 * q + half + 1) * 512]

            def hslice(hq, kb):
                return hq[:].rearrange("p (kb dr pb) -> p kb dr pb",
                                       kb=KB, dr=2)[:, kb, :, 0:BS]

            def hq_view(hq):
                return hq[:].rearrange("p (kb dr pb) -> p kb dr pb",
                                       kb=KB, dr=2)[:, :, :, 0:BS]

            def eslice(t):
                return embq[:].rearrange("p (dr ts pb) -> p dr ts pb",
                                         ts=TS, pb=PB)[:, :, t, 0:BS]

            # ---------------- pre-phase ----------------
            with tc.tile_pool(name="pre_ps", bufs=2, space="PSUM") as pre_ps, \
                 tc.tile_pool(name="prew_ps", bufs=2, space="PSUM") as prew_ps, \
                 tc.tile_pool(name="pre_sb", bufs=2) as pre_sb:
                # token embedding gather -> embq (dr row 0), label E rows
                for rt, (r0, nr) in enumerate(((0, 128), (128, NROW - 128))):
                    t0, t1_ = (0, 32) if rt == 0 else (32, TS)
                    idx = pre_sb.tile([128, 1], I32, tag="idx")
                    nc.sync.dma_start(idx[:nr], tokidx[r0:r0 + nr, :])
                    eg = pre_sb.tile([128, RANK], F32, tag="eg")
                    nc.gpsimd.indirect_dma_start(
                        out=eg[:nr], out_offset=None, in_=e_rows[:],
                        in_offset=bass.IndirectOffsetOnAxis(ap=idx[:nr, :1], axis=0))
                    ps = pre_ps.tile([128, 128], F32, tag="tr")
                    nc.tensor.transpose(ps[:, :nr], eg[:nr, :], id4[:nr, :nr])
                    nc.vector.tensor_copy(
                        embq[:].rearrange("p (dr ts pb) -> p dr ts pb", ts=TS, pb=PB)
                        [:, 0, t0:t1_, 0:BS],
                        ps[:, :nr].rearrange("p (ts pb) -> p ts pb", pb=BS))
                    idx2 = pre_sb.tile([128, 1], I32, tag="idx2")
                    nc.sync.dma_start(idx2[:nr], labidx[r0:r0 + nr, :])
                    nc.gpsimd.indirect_dma_start(
                        out=elab[:nr, rt * RANK:(rt + 1) * RANK],
                        out_offset=None, in_=e_rows[:],
                        in_offset=bass.IndirectOffsetOnAxis(ap=idx2[:nr, :1], axis=0))

                # encW = (enc @ W0c) * WS (stacked pairs), encWo = enc @ Wo[H:]
                for n in range(8):
                    wck = pre_sb.tile([128, 8 * 512], BF16, tag="wck")
                    nc.sync.dma_start(
                        wck[:].rearrange("p (k n) -> p k n", k=8),
                        wc[:, n * 512:(n + 1) * 512]
                        .rearrange("(k p) n -> p k n", p=128))
                    for pair in range(2):
                        pw = prew_ps.tile([128, 512], F32, tag="pw")
                        for k in range(8):
                            nc.tensor.matmul(
                                pw[:], encp(k, pair),
                                wck[:, k * 512:(k + 1) * 512],
                                start=(k == 0), stop=(k == 7))
                        nc.scalar.mul(
                            encW[pair][:, n * 512:(n + 1) * 512], pw[:], WS)
                for pair in range(2):
                    pwo = prew_ps.tile([128, 512], F32, tag="pw")
                    for k in range(8):
                        wok = pre_sb.tile([128, 512], BF16, tag="wok")
                        nc.sync.dma_start(wok[:],
                                          wo[H + k * 128:H + (k + 1) * 128, :])
                        nc.tensor.matmul(pwo[:], encp(k, pair), wok[:],
                                         start=(k == 0), stop=(k == 7))
                    nc.vector.tensor_copy(encWo[pair][:], pwo[:])

            # ---------------- scan + interleaved CE ----------------
            with tc.tile_pool(name="pg", bufs=2, space="PSUM") as pg, \
                 tc.tile_pool(name="ptr", bufs=1, space="PSUM") as ptr, \
                 tc.tile_pool(name="plg", bufs=1, space="PSUM") as plg, \
                 tc.tile_pool(name="pce", bufs=2, space="PSUM") as pce, \
                 tc.tile_pool(name="sb", bufs=1) as sb, \
                 tc.tile_pool(name="sbs", bufs=2) as sbs, \
                 tc.tile_pool(name="ebuf", bufs=4) as ebuf:

                def quarter_l0(t, q, hook=None):
                    gp = pg.tile([BS, 1024], F32, tag="g")
                    ops = ([(eslice(t), 0)] +
                           [(hslice(h0q, kb), kb + 1) for kb in range(KB)])
                    for i, (lh, _) in enumerate(ops):
                        last = (t == 0) and (i == len(ops) - 1)
                        nc.tensor.matmul(gp[:, 0:512], lh,
                                         wslice(w0e_s if i == 0 else u0_s,
                                                1 if i == 0 else 4,
                                                0 if i == 0 else i - 1, q, 0),
                                         start=(i == 0), stop=last,
                                         perf_mode=PM.DoubleRow)
                        nc.tensor.matmul(gp[:, 512:1024], lh,
                                         wslice(w0e_s if i == 0 else u0_s,
                                                1 if i == 0 else 4,
                                                0 if i == 0 else i - 1, q, 1),
                                         start=(i == 0), stop=last,
                                         perf_mode=PM.DoubleRow)
                    if hook is not None:
                        hook()
                    if t > 0:
                        pcol = (t - 1) * BS
                        for half in range(2):
                            nsl = slice((2 * q + half) * 512,
                                        (2 * q + half + 1) * 512)
                            for pair in range(2):
                                nc.tensor.matmul(
                                    gp[:, half * 512:(half + 1) * 512],
                                    pTce(pair)[:, pcol:pcol + 4],
                                    encW[pair][:, nsl],
                                    start=False, stop=(pair == 1))
                    return gp

                def quarter_l1_u(q):
                    gp = pg.tile([BS, 1024], F32, tag="g")
                    for kb in range(KB):
                        for half in range(2):
                            nc.tensor.matmul(gp[:, half * 512:(half + 1) * 512],
                                             hslice(h1q, kb),
                                             wslice(u1_s, 4, kb, q, half),
                                             start=(kb == 0), stop=False,
                                             perf_mode=PM.DoubleRow)
                    return gp

                def quarter_l1_w(gp, q):
                    for kb in range(KB):
                        for half in range(2):
                            nc.tensor.matmul(gp[:, half * 512:(half + 1) * 512],
                                             hslice(h0q, kb),
                                             wslice(w1_s, 4, kb, q, half),
                                             start=False,
                                             stop=(kb == KB - 1),
                                             perf_mode=PM.DoubleRow)

                def lstm(mk_quarter, cst):
                    """4 scratch tags a/b/cc/d shared by both layers."""
                    inv = 1.0 / WS
                    gi = mk_quarter(0)
                    si = sb.tile([BS, H], F32, tag="a")
                    nc.scalar.activation(si[:], gi[:], AF.Sigmoid, scale=inv)
                    gf = mk_quarter(1)
                    sf = sb.tile([BS, H], F32, tag="b")
                    nc.scalar.activation(sf[:], gf[:], AF.Sigmoid, scale=inv)
                    t1 = sb.tile([BS, H], F32, tag="cc")
                    nc.vector.tensor_mul(t1[:], sf[:], cst[:])
                    gg = mk_quarter(2)
                    tg = sb.tile([BS, H], F32, tag="b")
                    nc.scalar.activation(tg[:], gg[:], AF.Tanh, scale=inv)
                    t2 = sb.tile([BS, H], F32, tag="d")
                    nc.vector.tensor_mul(t2[:], si[:], tg[:])
                    nc.vector.tensor_add(cst[:], t1[:], t2[:])
                    tch = sb.tile([BS, H], F32, tag="b")
                    nc.scalar.activation(tch[:], cst[:], AF.Tanh)
                    go = mk_quarter(3)
                    so = sb.tile([BS, H], F32, tag="d")
                    nc.scalar.activation(so[:], go[:], AF.Sigmoid, scale=inv)
                    hn = sb.tile([BS, H], F32, tag="a")
                    nc.vector.tensor_mul(hn[:], so[:], tch[:])
                    return hn

                def transpose_h(hn):
                    hp = ptr.tile([128, NCH * BS], F32, tag="hp")
                    for ch in range(NCH):
                        nc.tensor.transpose(hp[:, ch * BS:(ch + 1) * BS],
                                            hn[:, ch * 128:(ch + 1) * 128],
                                            id4[:BS, :BS])
                    return hp

                def emit_ce_rowtile(rt):
                    r0 = rt * 128
                    nr = min(128, NROW - r0)
                    hps = plg.tile([128, 512], F32, tag="lg")
                    hpT = sbs.tile([128, 4 * 128], BF16, tag="hpT")
                    for m in range(4):
                        for kk in range(NCH):
                            wo_t = ebuf.tile([128, 128], BF16, tag="wot",
                                             bufs=2)
                            nc.sync.dma_start(
                                wo_t[:],
                                wo[kk * 128:(kk + 1) * 128,
                                   m * 128:(m + 1) * 128])
                            nc.tensor.matmul(
                                hps[:, m * 128:m * 128 + nr],
                                wo_t[:],
                                featsT[:, kk * NROW + r0:kk * NROW + r0 + nr],
                                start=(kk == 0), stop=False)
                        for pair in range(2):
                            nc.tensor.matmul(
                                hps[:, m * 128:m * 128 + nr],
                                encWo[pair][:, m * 128:(m + 1) * 128],
                                pTce(pair)[:, r0:r0 + nr],
                                start=False, stop=(pair == 1))
                        nc.scalar.activation(hpT[:, m * 128:m * 128 + nr],
                                             hps[:, m * 128:m * 128 + nr],
                                             AF.Tanh, bias=boT_s[:, m:m + 1])
                    qps = plg.tile([128, 512], F32, tag="lg")
                    for kk in range(4):
                        nc.tensor.matmul(qps[:, :nr],
                                         pT_s[:, kk * RANK:(kk + 1) * RANK],
                                         hpT[:, kk * 128:kk * 128 + nr],
                                         start=(kk == 0), stop=(kk == 3))
                    qeT = sbs.tile([RANK, 128], BF16, tag="qeT")
                    nc.scalar.copy(qeT[:, :nr], qps[:, :nr])
                    qef = sbs.tile([RANK, 128], F32, tag="qef")
                    nc.scalar.copy(qef[:, :nr], qps[:, :nr])
                    qtp = plg.tile([128, 512], F32, tag="lg")
                    nc.tensor.transpose(qtp[:nr, :RANK], qef[:, :nr], id4[:, :])
                    qe = sbs.tile([128, RANK], F32, tag="qe")
                    nc.scalar.copy(qe[:nr, :], qtp[:nr, :RANK])
                    lt = sbs.tile([128, RANK], F32, tag="lt")
                    nc.vector.tensor_mul(lt[:nr, :], qe[:nr, :],
                                         elab[:nr, rt * RANK:(rt + 1) * RANK])
                    nc.vector.reduce_sum(lab_ll[:nr, rt:rt + 1], lt[:nr, :],
                                         axis=AX.X)
                    off = 0
                    first = True
                    for vc in VCHUNKS:
                        et = ebuf.tile([RANK, 512], BF16, tag="et")
                        nc.sync.dma_start(et[:, :vc], eT[:, off:off + vc])
                        ps = pce.tile([128, 512], F32, tag="vce")
                        nc.tensor.matmul(ps[:nr, :vc], qeT[:, :nr], et[:, :vc],
                                         start=True, stop=True)
                        ex = sbs.tile([128, 512], BF16, tag="ex")
                        pexp = sbs.tile([128, 1], F32, tag="pexp")
                        nc.scalar.activation(ex[:nr, :vc], ps[:nr, :vc], AF.Exp,
                                             accum_out=pexp[:nr, :])
                        if first:
                            nc.vector.tensor_copy(sum_e[:nr, rt:rt + 1],
                                                  pexp[:nr, :])
                            first = False
                        else:
                            nc.vector.tensor_add(sum_e[:nr, rt:rt + 1],
                                                 sum_e[:nr, rt:rt + 1],
                                                 pexp[:nr, :])
                        off += vc

                def lstm_l1(h0n, cst):
                    """Layer-1 LSTM with the h0 transposes interleaved after
                    the U-parts of quarters 0/1 so the PE never stalls on the
                    L0 tail."""
                    inv = 1.0 / WS
                    g0 = quarter_l1_u(0)
                    g1 = quarter_l1_u(1)
                    hp0 = transpose_h(h0n)
                    nc.vector.tensor_copy(
                        hq_view(h0q),
                        hp0[:].rearrange("p (kb dr b) -> p kb dr b",
                                         kb=KB, dr=2))
                    quarter_l1_w(g0, 0)
                    quarter_l1_w(g1, 1)
                    si = sb.tile([BS, H], F32, tag="a")
                    nc.scalar.activation(si[:], g0[:], AF.Sigmoid, scale=inv)
                    g2 = quarter_l1_u(2)
                    quarter_l1_w(g2, 2)
                    sf = sb.tile([BS, H], F32, tag="b")
                    nc.scalar.activation(sf[:], g1[:], AF.Sigmoid, scale=inv)
                    t1 = sb.tile([BS, H], F32, tag="cc")
                    nc.vector.tensor_mul(t1[:], sf[:], cst[:])
                    g3 = quarter_l1_u(3)
                    quarter_l1_w(g3, 3)
                    tg = sb.tile([BS, H], F32, tag="b")
                    nc.scalar.activation(tg[:], g2[:], AF.Tanh, scale=inv)
                    t2 = sb.tile([BS, H], F32, tag="d")
                    nc.vector.tensor_mul(t2[:], si[:], tg[:])
                    nc.vector.tensor_add(cst[:], t1[:], t2[:])
                    tch = sb.tile([BS, H], F32, tag="b")
                    nc.scalar.activation(tch[:], cst[:], AF.Tanh)
                    so = sb.tile([BS, H], F32, tag="d")
                    nc.scalar.activation(so[:], g3[:], AF.Sigmoid, scale=inv)
                    hn = sb.tile([BS, H], F32, tag="a")
                    nc.vector.tensor_mul(hn[:], so[:], tch[:])
                    return hn

                pend = {}

                def finish_attention():
                    tt = pend.pop("t")
                    sc_all = pend.pop("sc")
                    scm = sbs.tile([BS, BS * S], F32, tag="scm")
                    nc.vector.tensor_mul(scm[:], sc_all[:], dmask_s[:])
                    sca = sbs.tile([BS, 2 * S], F32, tag="sca")
                    nc.vector.tensor_add(sca[:], scm[:, 0:2 * S],
                                         scm[:, 2 * S:4 * S])
                    scb = sbs.tile([BS, S], F32, tag="scb")
                    nc.vector.tensor_add(scb[:], sca[:, 0:S], sca[:, S:2 * S])
                    pe_ = sbs.tile([BS, S], F32, tag="pe")
                    ssum = sbs.tile([BS, 1], F32, tag="ssum")
                    nc.scalar.activation(pe_[:], scb[:], AF.Exp,
                                         accum_out=ssum[:])
                    sv = sbs.tile([BS, 1], F32, tag="sv")
                    nc.vector.tensor_add(sv[:], ssum[:], smaskT_s[:])
                    rs = sbs.tile([BS, 1], F32, tag="rs")
                    nc.vector.reciprocal(rs[:], sv[:])
                    pbf = sbs.tile([BS, S], F32, tag="pbf")
                    nc.vector.tensor_scalar_mul(pbf[:], pe_[:], rs[:, :])
                    ptp4 = ptr.tile([128, BS], F32, tag="hp")
                    nc.tensor.transpose(ptp4[0:S, :], pbf[:], id4[:BS, :BS])
                    pstg = sbs.tile([S, BS], BF16, tag="pstg")
                    nc.vector.tensor_copy(pstg[:], ptp4[0:S, :])
                    col = tt * BS
                    for pair in range(2):
                        nc.vector.tensor_copy(
                            pTce(pair)[0:S, col + 2 * pair:col + 2 * pair + 1],
                            pstg[:, 2 * pair:2 * pair + 1])
                        nc.sync.dma_start(
                            pTce(pair)[SPAD:SPAD + S,
                                       col + 2 * pair + 1:col + 2 * pair + 2],
                            pstg[:, 2 * pair + 1:2 * pair + 2])

                for t in range(TS):
                    # ---- layer 0 (+ deferred attention finish in q0) ----
                    hook = finish_attention if pend else None
                    h0n = lstm(
                        lambda q, t=t, hook=hook: quarter_l0(
                            t, q, hook if q == 0 else None), c0)

                    # ---- layer 1 (transposes interleaved) ----
                    h1n = lstm_l1(h0n, c1)
                    hp1 = transpose_h(h1n)
                    nc.vector.tensor_copy(
                        hq_view(h1q),
                        hp1[:].rearrange("p (kb dr b) -> p kb dr b",
                                         kb=KB, dr=2))
                    nc.vector.tensor_copy(
                        featsT[:].rearrange("p (c r) -> p c r", c=NCH)
                        [:, :, t * BS:(t + 1) * BS],
                        hp1[:].rearrange("p (c b) -> p c b", c=NCH))

                    # ---- attention score MMs; softmax finish deferred ----
                    sc_all = ptr.tile([BS, BS * S], F32, tag="hp")
                    for j in range(BS):
                        for ch in range(NCH):
                            nc.tensor.matmul(
                                sc_all[:, j * S:(j + 1) * S],
                                featsT[:].rearrange("p (c r) -> p c r", c=NCH)
                                [:, ch, t * BS:(t + 1) * BS],
                                encj(ch, j),
                                start=(ch == 0), stop=(ch == NCH - 1))
                    pend["t"] = t
                    pend["sc"] = sc_all

                    if t == 33:
                        emit_ce_rowtile(0)

                finish_attention()
                emit_ce_rowtile(1)

                # ---- finalize partial loss (no collective) ----
                lse = sbs.tile([128, 2], F32, tag="lse")
                nc.scalar.activation(lse[:], sum_e[:], AF.Ln)
                nll = sbs.tile([128, 2], F32, tag="nll")
                nc.vector.tensor_sub(nll[:], lse[:], lab_ll[:])
                nllm = sbs.tile([128, 2], F32, tag="nllm")
                nc.vector.tensor_mul(nllm[:], nll[:], vmask_s[:])
                lp = ptr.tile([1, 2], F32, tag="hp")
                nc.tensor.matmul(lp[:], ones128[:, :], nllm[:],
                                 start=True, stop=True)
                lsum = sbs.tile([1, 1], F32, tag="lsum")
                nc.vector.reduce_sum(lsum[:], lp[:], axis=AX.X)
                nc.sync.dma_start(out_loss[:], lsum[:])

    nc.compile()
    return nc


def _quant_w(W, kb):
    """W [K, 4096] f32 -> fp8 [128, kb*2*8*512] in (kb, dr, nc, n) layout,
    scaled by WS."""
    K = W.shape[0]
    assert K == kb * 256
    Wq = (W * WS).reshape(kb, 2, 128, 8, 512)       # [kb, dr, p, nc, n]
    Wq = Wq.transpose(2, 0, 1, 3, 4).reshape(128, kb * 2 * 8 * 512)
    return np.ascontiguousarray(Wq).astype(FP8NP)


def _prep_inputs(inputs):
    f32 = np.float32
    enc = np.asarray(inputs["encoded"], f32)
    est = np.asarray(inputs["encoder_state"], f32)
    tok = np.asarray(inputs["tgt_tokens"]).astype(np.int32)
    enc_lens = np.asarray(inputs["enc_lens"]).astype(np.int32)
    tgt_lens = np.asarray(inputs["tgt_lens"]).astype(np.int32)
    E = np.asarray(inputs["E"], f32)
    P = np.asarray(inputs["P"], f32)
    W0 = np.asarray(inputs["W0"], f32)
    U0 = np.asarray(inputs["U0"], f32)
    W1 = np.asarray(inputs["W1"], f32)
    U1 = np.asarray(inputs["U1"], f32)
    Wo = np.asarray(inputs["Wo"], f32)
    bo = np.asarray(inputs["bo"], f32)
    b0 = np.asarray(inputs["b0"], f32)
    b1 = np.asarray(inputs["b1"], f32)
    assert not b0.any() and not b1.any(), "nonzero LSTM bias unsupported"

    eT = np.ascontiguousarray(E.T).astype(BF16NP)
    pTb = np.ascontiguousarray(P.T).astype(BF16NP)
    wo_b = Wo.astype(BF16NP)
    wc_b = np.ascontiguousarray(W0[EMB:]).astype(BF16NP)
    boT = np.ascontiguousarray(bo.reshape(4, 128).T)          # [128, 4]

    # fold P into the embedding input weights; pad K 128 -> 256 with zeros
    w0e_f = np.zeros((256, 4 * H), f32)
    w0e_f[:RANK] = P @ W0[:EMB]
    w0e_q = _quant_w(w0e_f, 1)
    u0_q = _quant_w(U0, 4)
    w1_q = _quant_w(W1, 4)
    u1_q = _quant_w(U1, 4)

    def hq_init(h):   # h [BS, H] -> fp8 [128, (kb, dr, PB)] padded
        ht = h.T.reshape(KB, 2, 128, BS).transpose(2, 0, 1, 3)
        out = np.zeros((128, KB, 2, PB), np.float32)
        out[:, :, :, :BS] = ht
        return np.ascontiguousarray(out.reshape(128, KB * 2 * PB)).astype(FP8NP)

    in_maps = []
    for k in range(NCORES):
        ob = slice(k * BS, (k + 1) * BS)
        encz = enc[:, ob, :].copy()
        pad = np.arange(S)[:, None] >= enc_lens[ob][None, :]   # [S, BS]
        encz[pad, :] = 0.0
        encTz = encz.transpose(2, 1, 0)                        # [C, BS, S]
        # pair-padded layout [128, (ch, pair, 2*SPAD)]
        encTP_o = np.zeros((C, 2, 2 * SPAD), f32)
        for b in range(BS):
            encTP_o[:, b // 2, (b % 2) * SPAD:(b % 2) * SPAD + S] = \
                encTz[:, b, :]
        encTP_o = np.ascontiguousarray(
            encTP_o.reshape(NCH, 128, 2 * 2 * SPAD)
            .transpose(1, 0, 2).reshape(128, NCH * 2 * 2 * SPAD)).astype(BF16NP)
        sm = -(S - enc_lens[ob]).astype(f32).reshape(BS, 1)
        dm = np.zeros((BS, BS * S), f32)
        for j in range(BS):
            dm[j, j * S:(j + 1) * S] = 1.0
        tokid = np.ascontiguousarray(
            tok[ob, :TS].T.reshape(NROW, 1)).astype(np.int32)
        lab = np.ascontiguousarray(
            tok[ob, 1:T].T.reshape(NROW, 1)).astype(np.int32)
        vm = (np.arange(TS)[:, None] <
              (tgt_lens[ob] - 1)[None, :]).astype(f32).reshape(NROW)
        vmp = np.zeros((128, 2), f32)
        vmp[:128, 0] = vm[:128]
        vmp[:NROW - 128, 1] = vm[128:]
        in_maps.append({
            "w0e": w0e_q, "u0": u0_q, "w1": w1_q, "u1": u1_q,
            "wc": wc_b, "wo": wo_b, "boT": boT, "pTb": pTb,
            "encTP": encTP_o, "smaskT": sm, "dmask": dm,
            "h0q_i": hq_init(est[0, ob]), "h1q_i": hq_init(est[1, ob]),
            "e_rows": E, "eT": eT,
            "tokidx": tokid, "labidx": lab, "vmask": vmp,
        })
    return in_maps


_NC_CACHE = {}


def kernel(**inputs) -> np.ndarray:
    if "nc" not in _NC_CACHE:
        _NC_CACHE["nc"] = build()
    nc = _NC_CACHE["nc"]
    in_maps = _prep_inputs(inputs)
    res = bass_utils.run_bass_kernel_spmd(
        nc, in_maps, core_ids=list(range(NCORES)))
    _NC_CACHE["res"] = res
    total = sum(np.float32(res.results[c]["loss"][0, 0])
                for c in range(NCORES))
    return np.float32(total)


# revision 18
# speedup vs baseline: 1.0111x; 1.0111x over previous
"""Trainium2 Bass kernel for nn_Decoder — data-parallel, zero-collective design.

Each core owns 4 batches end-to-end:
  - 2-layer LSTM scan, weights resident in fp8-e4m3 (scaled x8, compensated
    in ACT scale), gates in [4,1024] PSUM quarters via DoubleRow fp8 matmuls.
  - P@W0[:EMB] folded host-side: the embedding input contributes one padded
    256-DR chunk (K=128 real) instead of two (K=512).
  - ctx contribution algebraically moved to attention weights:
    gates_ctx = p_att @ (enc @ W0c).  encW = enc@W0c is precomputed ON DEVICE
    once (enc is step-invariant), so the per-step ctx K drops 1024 -> 128
    (bf16 block-diag p stationaries; batch pair packed at partition rows
    0-55 / 64-119 so PE transposes can produce them directly via the
    tile_position col-offset).
  - Attention softmax batched over the 4 batches ([4,56] one exp chain).
  - CE over full vocab for own 188 (t,b) rows; ctx part of hproj uses
    encWo = enc@Wo_ctx (also precomputed) with the stored p columns.
    Partial loss written out; host sums the 8 partials. NO collectives.
"""
import sys
sys.path.insert(0, '/opt/trn_rl_repo')

import numpy as np
import ml_dtypes
import concourse.bass as bass
import concourse.bacc as bacc
import concourse.tile as tile
import concourse.mybir as mybir
from concourse import bass_utils
from concourse.masks import make_identity

AF = mybir.ActivationFunctionType
ALU = mybir.AluOpType
AX = mybir.AxisListType
PM = mybir.MatmulPerfMode
F32 = mybir.dt.float32
BF16 = mybir.dt.bfloat16
FP8 = mybir.dt.float8e4
I32 = mybir.dt.int32

NCORES = 8
S, B, C = 56, 32, 1024
T = 48
H = 1024
EMB = 512
RANK = 128
VOCAB = 32000
TS = T - 1          # 47
BS = B // NCORES    # 4
NCH = 8             # 128-chunks of H/C
KB = 4              # 256-blocks of H (DoubleRow K-chunks)
NROW = TS * BS      # 188
PB = 16             # padded batch block in fp8 DR state (16B stride align)
SPAD = 64           # padded per-batch row block in pair stacking
WS = 8.0            # fp8 weight pre-scale
VCHUNKS = [512] * 62 + [256]
assert sum(VCHUNKS) == VOCAB

BF16NP = ml_dtypes.bfloat16
FP8NP = ml_dtypes.float8_e4m3


def build():
    nc = bacc.Bacc("TRN2", target_bir_lowering=False, debug=False,
                   num_devices=NCORES)

    def din(name, shape, dt):
        return nc.dram_tensor(name, shape, dt, kind="ExternalInput").ap()

    # fp8 weights, host layout [128, (kb, dr, nc, 512)]
    w0e = din("w0e", [128, 1 * 2 * 4096], FP8)   # P@W0[:EMB] folded, K padded
    u0 = din("u0", [128, 4 * 2 * 4096], FP8)
    w1 = din("w1", [128, 4 * 2 * 4096], FP8)
    u1 = din("u1", [128, 4 * 2 * 4096], FP8)
    wc = din("wc", [H, 4 * H], BF16)             # W0[EMB:], for encW precompute
    wo = din("wo", [H + C, EMB], BF16)
    boT = din("boT", [128, 4], F32)
    pTb = din("pTb", [EMB, RANK], BF16)
    # enc hidden-major, pair-padded: [128, (ch, pair, 2*SPAD)]
    encTP = din("encTP", [128, NCH * 2 * 2 * SPAD], BF16)
    smaskT = din("smaskT", [BS, 1], F32)
    dmask = din("dmask", [BS, BS * S], F32)
    h0q_i = din("h0q_i", [128, KB * 2 * PB], FP8)
    h1q_i = din("h1q_i", [128, KB * 2 * PB], FP8)
    e_rows = din("e_rows", [VOCAB, RANK], F32)
    eT = din("eT", [RANK, VOCAB], BF16)
    tokidx = din("tokidx", [NROW, 1], I32)
    labidx = din("labidx", [NROW, 1], I32)
    vmask = din("vmask", [128, 2], F32)

    out_loss = nc.dram_tensor("loss", [1, 1], F32, kind="ExternalOutput").ap()

    with tile.TileContext(nc, num_cores=NCORES) as tc:
        with tc.tile_pool(name="consts", bufs=1) as consts, \
             tc.tile_pool(name="wpool", bufs=1) as wpool, \
             tc.tile_pool(name="state", bufs=1) as state:

            id4 = consts.tile([128, 128], F32)
            make_identity(nc, id4)
            ones128 = consts.tile([128, 1], F32)
            nc.gpsimd.memset(ones128[:], 1.0)

            def wtile(ap, shape, dt, tag):
                t_ = wpool.tile(shape, dt, tag=tag)
                nc.sync.dma_start(t_[:], ap[:])
                return t_

            def wtile_ch(ap, nchunk, width, dt, tag):
                t_ = wpool.tile([128, nchunk * width], dt, tag=tag)
                nc.sync.dma_start(
                    t_[:].rearrange("p (c g) -> p c g", c=nchunk),
                    ap[:].rearrange("(c p) g -> p c g", p=128))
                return t_

            w0e_s = wtile(w0e, [128, 1 * 2 * 4096], FP8, tag="w0e_s")
            u0_s = wtile(u0, [128, 4 * 2 * 4096], FP8, tag="u0_s")
            w1_s = wtile(w1, [128, 4 * 2 * 4096], FP8, tag="w1_s")
            u1_s = wtile(u1, [128, 4 * 2 * 4096], FP8, tag="u1_s")
            boT_s = wtile(boT, [128, 4], F32, tag="boT_s")
            pT_s = wtile_ch(pTb, 4, RANK, BF16, tag="pT_s")
            encTP_s = wtile(encTP, [128, NCH * 2 * 2 * SPAD], BF16,
                            tag="encTP_s")
            smaskT_s = wtile(smaskT, [BS, 1], F32, tag="smaskT_s")
            dmask_s = wtile(dmask, [BS, BS * S], F32, tag="dmask_s")
            vmask_s = wtile(vmask, [128, 2], F32, tag="vmask_s")

            def encp(ch, pair):        # [128, 128] pair-padded enc cols
                return encTP_s[:].rearrange("p (c q s) -> p c q s",
                                            c=NCH, q=2)[:, ch, pair, :]

            def encj(ch, j):           # [128, S] single-batch enc cols
                return encTP_s[:].rearrange("p (c q s) -> p c q s",
                                            c=NCH, q=2)[:, ch, j // 2,
                                                        (j % 2) * SPAD:
                                                        (j % 2) * SPAD + S]

            # fp8 DR-layout state [128, (kb, dr, PB)], batch block padded
            h0q = state.tile([128, KB * 2 * PB], FP8)
            nc.sync.dma_start(h0q[:], h0q_i[:])
            h1q = state.tile([128, KB * 2 * PB], FP8)
            nc.sync.dma_start(h1q[:], h1q_i[:])
            c0 = state.tile([BS, H], F32)
            nc.gpsimd.memset(c0[:], 0.0)
            c1 = state.tile([BS, H], F32)
            nc.gpsimd.memset(c1[:], 0.0)

            # embedding stationary, DR layout [128, (dr, t, PB)], dr row 1 zero
            embq = state.tile([128, 2 * TS * PB], FP8)
            nc.gpsimd.memset(embq[:], 0.0)
            featsT = state.tile([128, NCH * NROW], BF16)   # h1 hidden-major
            elab = state.tile([128, 2 * RANK], F32)
            sum_e = state.tile([128, 2], F32)
            nc.gpsimd.memset(sum_e[:], 1.0)
            lab_ll = state.tile([128, 2], F32)
            nc.gpsimd.memset(lab_ll[:], 0.0)

            # stored attention p columns (pair-packed rows), used BOTH as the
            # per-step ctx-gates stationary and the CE ctx moving operand.
            pTce_all = state.tile([128, 2 * (NROW + 4)], BF16)
            nc.gpsimd.memset(pTce_all[:], 0.0)

            def pTce(pair):
                return pTce_all[:].rearrange("p (q c) -> p q c",
                                             q=2)[:, pair, :]

            # enc@W0c (x WS) and enc@Wo_ctx, pair-packed rows
            encW = [state.tile([128, 4 * H], BF16, tag=f"encW{i}",
                               name=f"encW{i}") for i in range(2)]
            encWo = [state.tile([128, EMB], BF16, tag=f"encWo{i}",
                                name=f"encWo{i}") for i in range(2)]

            def wslice(w, nkb, kb, q, half):
                # [128, (kb dr n)] -> [128, 2, 512] for N-chunk 2q+half
                return w[:].rearrange("p (kb dr n) -> p kb dr n",
                                      kb=nkb, dr=2)[:, kb, :,
                                                    (2 * q + half) * 512:
                                                    (2 * q + half + 1) * 512]

            def hslice(hq, kb):
                return hq[:].rearrange("p (kb dr pb) -> p kb dr pb",
                                       kb=KB, dr=2)[:, kb, :, 0:BS]

            def hq_view(hq):
                return hq[:].rearrange("p (kb dr pb) -> p kb dr pb",
                                       kb=KB, dr=2)[:, :, :, 0:BS]

            def eslice(t):
                return embq[:].rearrange("p (dr ts pb) -> p dr ts pb",
                                         ts=TS, pb=PB)[:, :, t, 0:BS]

            # ---------------- pre-phase ----------------
            with tc.tile_pool(name="pre_ps", bufs=2, space="PSUM") as pre_ps, \
                 tc.tile_pool(name="prew_ps", bufs=2, space="PSUM") as prew_ps, \
                 tc.tile_pool(name="pre_sb", bufs=2) as pre_sb:
                # token embedding gather -> embq (dr row 0), label E rows
                for rt, (r0, nr) in enumerate(((0, 128), (128, NROW - 128))):
                    t0, t1_ = (0, 32) if rt == 0 else (32, TS)
                    idx = pre_sb.tile([128, 1], I32, tag="idx")
                    nc.sync.dma_start(idx[:nr], tokidx[r0:r0 + nr, :])
                    eg = pre_sb.tile([128, RANK], F32, tag="eg")
                    nc.gpsimd.indirect_dma_start(
                        out=eg[:nr], out_offset=None, in_=e_rows[:],
                        in_offset=bass.IndirectOffsetOnAxis(ap=idx[:nr, :1], axis=0))
                    ps = pre_ps.tile([128, 128], F32, tag="tr")
                    nc.tensor.transpose(ps[:, :nr], eg[:nr, :], id4[:nr, :nr])
                    nc.vector.tensor_copy(
                        embq[:].rearrange("p (dr ts pb) -> p dr ts pb", ts=TS, pb=PB)
                        [:, 0, t0:t1_, 0:BS],
                        ps[:, :nr].rearrange("p (ts pb) -> p ts pb", pb=BS))
                    idx2 = pre_sb.tile([128, 1], I32, tag="idx2")
                    nc.sync.dma_start(idx2[:nr], labidx[r0:r0 + nr, :])
                    nc.gpsimd.indirect_dma_start(
                        out=elab[:nr, rt * RANK:(rt + 1) * RANK],
                        out_offset=None, in_=e_rows[:],
                        in_offset=bass.IndirectOffsetOnAxis(ap=idx2[:nr, :1], axis=0))

                # encW = (enc @ W0c) * WS (stacked pairs), encWo = enc @ Wo[H:]
                for n in range(8):
                    wck = pre_sb.tile([128, 8 * 512], BF16, tag="wck")
                    nc.sync.dma_start(
                        wck[:].rearrange("p (k n) -> p k n", k=8),
                        wc[:, n * 512:(n + 1) * 512]
                        .rearrange("(k p) n -> p k n", p=128))
                    for pair in range(2):
                        pw = prew_ps.tile([128, 512], F32, tag="pw")
                        for k in range(8):
                            nc.tensor.matmul(
                                pw[:], encp(k, pair),
                                wck[:, k * 512:(k + 1) * 512],
                                start=(k == 0), stop=(k == 7))
                        nc.scalar.mul(
                            encW[pair][:, n * 512:(n + 1) * 512], pw[:], WS)
                for pair in range(2):
                    pwo = prew_ps.tile([128, 512], F32, tag="pw")
                    for k in range(8):
                        wok = pre_sb.tile([128, 512], BF16, tag="wok")
                        nc.sync.dma_start(wok[:],
                                          wo[H + k * 128:H + (k + 1) * 128, :])
                        nc.tensor.matmul(pwo[:], encp(k, pair), wok[:],
                                         start=(k == 0), stop=(k == 7))
                    nc.vector.tensor_copy(encWo[pair][:], pwo[:])

            # ---------------- scan + interleaved CE ----------------
            with tc.tile_pool(name="pg", bufs=2, space="PSUM") as pg, \
                 tc.tile_pool(name="ptr", bufs=1, space="PSUM") as ptr, \
                 tc.tile_pool(name="plg", bufs=1, space="PSUM") as plg, \
                 tc.tile_pool(name="pce", bufs=2, space="PSUM") as pce, \
                 tc.tile_pool(name="sb", bufs=1) as sb, \
                 tc.tile_pool(name="sbs", bufs=2) as sbs, \
                 tc.tile_pool(name="ebuf", bufs=4) as ebuf:

                def quarter_l0(t, q, hook=None):
                    gp = pg.tile([BS, 1024], F32, tag="g")
                    ops = ([(eslice(t), 0)] +
                           [(hslice(h0q, kb), kb + 1) for kb in range(KB)])
                    for i, (lh, _) in enumerate(ops):
                        last = (t == 0) and (i == len(ops) - 1)
                        nc.tensor.matmul(gp[:, 0:512], lh,
                                         wslice(w0e_s if i == 0 else u0_s,
                                                1 if i == 0 else 4,
                                                0 if i == 0 else i - 1, q, 0),
                                         start=(i == 0), stop=last,
                                         perf_mode=PM.DoubleRow)
                        nc.tensor.matmul(gp[:, 512:1024], lh,
                                         wslice(w0e_s if i == 0 else u0_s,
                                                1 if i == 0 else 4,
                                                0 if i == 0 else i - 1, q, 1),
                                         start=(i == 0), stop=last,
                                         perf_mode=PM.DoubleRow)
                    if hook is not None:
                        hook()
                    if t > 0:
                        pcol = (t - 1) * BS
                        for half in range(2):
                            nsl = slice((2 * q + half) * 512,
                                        (2 * q + half + 1) * 512)
                            for pair in range(2):
                                nc.tensor.matmul(
                                    gp[:, half * 512:(half + 1) * 512],
                                    pTce(pair)[:, pcol:pcol + 4],
                                    encW[pair][:, nsl],
                                    start=False, stop=(pair == 1))
                    return gp

                def quarter_l1_u(q):
                    gp = pg.tile([BS, 1024], F32, tag="g")
                    for kb in range(KB):
                        for half in range(2):
                            nc.tensor.matmul(gp[:, half * 512:(half + 1) * 512],
                                             hslice(h1q, kb),
                                             wslice(u1_s, 4, kb, q, half),
                                             start=(kb == 0), stop=False,
                                             perf_mode=PM.DoubleRow)
                    return gp

                def quarter_l1_w(gp, q):
                    for kb in range(KB):
                        for half in range(2):
                            nc.tensor.matmul(gp[:, half * 512:(half + 1) * 512],
                                             hslice(h0q, kb),
                                             wslice(w1_s, 4, kb, q, half),
                                             start=False,
                                             stop=(kb == KB - 1),
                                             perf_mode=PM.DoubleRow)

                def lstm(mk_quarter, cst):
                    """4 scratch tags a/b/cc/d shared by both layers."""
                    inv = 1.0 / WS
                    gi = mk_quarter(0)
                    si = sb.tile([BS, H], F32, tag="a")
                    nc.scalar.activation(si[:], gi[:], AF.Sigmoid, scale=inv)
                    gf = mk_quarter(1)
                    sf = sb.tile([BS, H], F32, tag="b")
                    nc.scalar.activation(sf[:], gf[:], AF.Sigmoid, scale=inv)
                    t1 = sb.tile([BS, H], F32, tag="cc")
                    nc.vector.tensor_mul(t1[:], sf[:], cst[:])
                    gg = mk_quarter(2)
                    tg = sb.tile([BS, H], F32, tag="b")
                    nc.scalar.activation(tg[:], gg[:], AF.Tanh, scale=inv)
                    t2 = sb.tile([BS, H], F32, tag="d")
                    nc.vector.tensor_mul(t2[:], si[:], tg[:])
                    nc.vector.tensor_add(cst[:], t1[:], t2[:])
                    tch = sb.tile([BS, H], F32, tag="b")
                    nc.scalar.activation(tch[:], cst[:], AF.Tanh)
                    go = mk_quarter(3)
                    so = sb.tile([BS, H], F32, tag="d")
                    nc.scalar.activation(so[:], go[:], AF.Sigmoid, scale=inv)
                    hn = sb.tile([BS, H], F32, tag="a")
                    nc.vector.tensor_mul(hn[:], so[:], tch[:])
                    return hn

                def transpose_h(hn):
                    hp = ptr.tile([128, NCH * BS], F32, tag="hp")
                    for ch in range(NCH):
                        nc.tensor.transpose(hp[:, ch * BS:(ch + 1) * BS],
                                            hn[:, ch * 128:(ch + 1) * 128],
                                            id4[:BS, :BS])
                    return hp

                def emit_ce_rowtile(rt):
                    r0 = rt * 128
                    nr = min(128, NROW - r0)
                    hps = plg.tile([128, 512], F32, tag="lg")
                    hpT = sbs.tile([128, 4 * 128], BF16, tag="hpT")
                    for m in range(4):
                        for kk in range(NCH):
                            wo_t = ebuf.tile([128, 128], BF16, tag="wot",
                                             bufs=2)
                            nc.sync.dma_start(
                                wo_t[:],
                                wo[kk * 128:(kk + 1) * 128,
                                   m * 128:(m + 1) * 128])
                            nc.tensor.matmul(
                                hps[:, m * 128:m * 128 + nr],
                                wo_t[:],
                                featsT[:, kk * NROW + r0:kk * NROW + r0 + nr],
                                start=(kk == 0), stop=False)
                        for pair in range(2):
                            nc.tensor.matmul(
                                hps[:, m * 128:m * 128 + nr],
                                encWo[pair][:, m * 128:(m + 1) * 128],
                                pTce(pair)[:, r0:r0 + nr],
                                start=False, stop=(pair == 1))
                        nc.scalar.activation(hpT[:, m * 128:m * 128 + nr],
                                             hps[:, m * 128:m * 128 + nr],
                                             AF.Tanh, bias=boT_s[:, m:m + 1])
                    qps = plg.tile([128, 512], F32, tag="lg")
                    for kk in range(4):
                        nc.tensor.matmul(qps[:, :nr],
                                         pT_s[:, kk * RANK:(kk + 1) * RANK],
                                         hpT[:, kk * 128:kk * 128 + nr],
                                         start=(kk == 0), stop=(kk == 3))
                    qeT = sbs.tile([RANK, 128], BF16, tag="qeT")
                    nc.scalar.copy(qeT[:, :nr], qps[:, :nr])
                    qef = sbs.tile([RANK, 128], F32, tag="qef")
                    nc.scalar.copy(qef[:, :nr], qps[:, :nr])
                    qtp = plg.tile([128, 512], F32, tag="lg")
                    nc.tensor.transpose(qtp[:nr, :RANK], qef[:, :nr], id4[:, :])
                    qe = sbs.tile([128, RANK], F32, tag="qe")
                    nc.scalar.copy(qe[:nr, :], qtp[:nr, :RANK])
                    lt = sbs.tile([128, RANK], F32, tag="lt")
                    nc.vector.tensor_mul(lt[:nr, :], qe[:nr, :],
                                         elab[:nr, rt * RANK:(rt + 1) * RANK])
                    nc.vector.reduce_sum(lab_ll[:nr, rt:rt + 1], lt[:nr, :],
                                         axis=AX.X)
                    off = 0
                    first = True
                    for vc in VCHUNKS:
                        et = ebuf.tile([RANK, 512], BF16, tag="et")
                        nc.sync.dma_start(et[:, :vc], eT[:, off:off + vc])
                        ps = pce.tile([128, 512], F32, tag="vce")
                        nc.tensor.matmul(ps[:nr, :vc], qeT[:, :nr], et[:, :vc],
                                         start=True, stop=True)
                        ex = sbs.tile([128, 512], BF16, tag="ex")
                        pexp = sbs.tile([128, 1], F32, tag="pexp")
                        nc.scalar.activation(ex[:nr, :vc], ps[:nr, :vc], AF.Exp,
                                             accum_out=pexp[:nr, :])
                        if first:
                            nc.vector.tensor_copy(sum_e[:nr, rt:rt + 1],
                                                  pexp[:nr, :])
                            first = False
                        else:
                            nc.vector.tensor_add(sum_e[:nr, rt:rt + 1],
                                                 sum_e[:nr, rt:rt + 1],
                                                 pexp[:nr, :])
                        off += vc

                def lstm_l1(h0n, cst):
                    """Layer-1 LSTM with the h0 transposes interleaved after
                    the U-parts of quarters 0/1 so the PE never stalls on the
                    L0 tail."""
                    inv = 1.0 / WS
                    g0 = quarter_l1_u(0)
                    g1 = quarter_l1_u(1)
                    hp0 = transpose_h(h0n)
                    nc.vector.tensor_copy(
                        hq_view(h0q),
                        hp0[:].rearrange("p (kb dr b) -> p kb dr b",
                                         kb=KB, dr=2))
                    quarter_l1_w(g0, 0)
                    quarter_l1_w(g1, 1)
                    si = sb.tile([BS, H], F32, tag="a")
                    nc.scalar.activation(si[:], g0[:], AF.Sigmoid, scale=inv)
                    g2 = quarter_l1_u(2)
                    quarter_l1_w(g2, 2)
                    sf = sb.tile([BS, H], F32, tag="b")
                    nc.scalar.activation(sf[:], g1[:], AF.Sigmoid, scale=inv)
                    t1 = sb.tile([BS, H], F32, tag="cc")
                    nc.vector.tensor_mul(t1[:], sf[:], cst[:])
                    g3 = quarter_l1_u(3)
                    quarter_l1_w(g3, 3)
                    tg = sb.tile([BS, H], F32, tag="b")
                    nc.scalar.activation(tg[:], g2[:], AF.Tanh, scale=inv)
                    t2 = sb.tile([BS, H], F32, tag="d")
                    nc.vector.tensor_mul(t2[:], si[:], tg[:])
                    nc.vector.tensor_add(cst[:], t1[:], t2[:])
                    tch = sb.tile([BS, H], F32, tag="b")
                    nc.scalar.activation(tch[:], cst[:], AF.Tanh)
                    so = sb.tile([BS, H], F32, tag="d")
                    nc.scalar.activation(so[:], g3[:], AF.Sigmoid, scale=inv)
                    hn = sb.tile([BS, H], F32, tag="a")
                    nc.vector.tensor_mul(hn[:], so[:], tch[:])
                    return hn

                pend = {}

                def finish_attention():
                    tt = pend.pop("t")
                    sc_all = pend.pop("sc")
                    scm = sbs.tile([BS, BS * S], F32, tag="scm")
                    nc.vector.tensor_mul(scm[:], sc_all[:], dmask_s[:])
                    sca = sbs.tile([BS, 2 * S], F32, tag="sca")
                    nc.vector.tensor_add(sca[:], scm[:, 0:2 * S],
                                         scm[:, 2 * S:4 * S])
                    scb = sbs.tile([BS, S], F32, tag="scb")
                    nc.vector.tensor_add(scb[:], sca[:, 0:S], sca[:, S:2 * S])
                    pe_ = sbs.tile([BS, S], F32, tag="pe")
                    ssum = sbs.tile([BS, 1], F32, tag="ssum")
                    nc.scalar.activation(pe_[:], scb[:], AF.Exp,
                                         accum_out=ssum[:])
                    sv = sbs.tile([BS, 1], F32, tag="sv")
                    nc.vector.tensor_add(sv[:], ssum[:], smaskT_s[:])
                    rs = sbs.tile([BS, 1], F32, tag="rs")
                    nc.vector.reciprocal(rs[:], sv[:])
                    pbf = sbs.tile([BS, S], F32, tag="pbf")
                    nc.vector.tensor_scalar_mul(pbf[:], pe_[:], rs[:, :])
                    ptp4 = ptr.tile([128, BS], F32, tag="hp")
                    nc.tensor.transpose(ptp4[0:S, :], pbf[:], id4[:BS, :BS])
                    pstg = sbs.tile([S, BS], BF16, tag="pstg")
                    nc.vector.tensor_copy(pstg[:], ptp4[0:S, :])
                    col = tt * BS
                    for pair in range(2):
                        nc.vector.tensor_copy(
                            pTce(pair)[0:S, col + 2 * pair:col + 2 * pair + 1],
                            pstg[:, 2 * pair:2 * pair + 1])
                        nc.sync.dma_start(
                            pTce(pair)[SPAD:SPAD + S,
                                       col + 2 * pair + 1:col + 2 * pair + 2],
                            pstg[:, 2 * pair + 1:2 * pair + 2])

                for t in range(TS):
                    # ---- layer 0 (+ deferred attention finish in q0) ----
                    hook = finish_attention if pend else None
                    h0n = lstm(
                        lambda q, t=t, hook=hook: quarter_l0(
                            t, q, hook if q == 0 else None), c0)

                    # ---- layer 1 (transposes interleaved) ----
                    h1n = lstm_l1(h0n, c1)
                    hp1 = transpose_h(h1n)
                    nc.vector.tensor_copy(
                        hq_view(h1q),
                        hp1[:].rearrange("p (kb dr b) -> p kb dr b",
                                         kb=KB, dr=2))
                    nc.vector.tensor_copy(
                        featsT[:].rearrange("p (c r) -> p c r", c=NCH)
                        [:, :, t * BS:(t + 1) * BS],
                        hp1[:].rearrange("p (c b) -> p c b", c=NCH))

                    # ---- attention score MMs; softmax finish deferred ----
                    sc_all = ptr.tile([BS, BS * S], F32, tag="hp")
                    for j in range(BS):
                        for ch in range(NCH):
                            nc.tensor.matmul(
                                sc_all[:, j * S:(j + 1) * S],
                                featsT[:].rearrange("p (c r) -> p c r", c=NCH)
                                [:, ch, t * BS:(t + 1) * BS],
                                encj(ch, j),
                                start=(ch == 0), stop=(ch == NCH - 1))
                    pend["t"] = t
                    pend["sc"] = sc_all

                    if t == 33:
                        emit_ce_rowtile(0)

                finish_attention()
                emit_ce_rowtile(1)

                # ---- finalize partial loss (no collective) ----
                lse = sbs.tile([128, 2], F32, tag="lse")
                nc.scalar.activation(lse[:], sum_e[:], AF.Ln)
                nll = sbs.tile([128, 2], F32, tag="nll")
                nc.vector.tensor_sub(nll[:], lse[:], lab_ll[:])
                nllm = sbs.tile([128, 2], F32, tag="nllm")
                nc.vector.tensor_mul(nllm[:], nll[:], vmask_s[:])
                lp = ptr.tile([1, 2], F32, tag="hp")
                nc.tensor.matmul(lp[:], ones128[:, :], nllm[:],
                                 start=True, stop=True)
                lsum = sbs.tile([1, 1], F32, tag="lsum")
                nc.vector.reduce_sum(lsum[:], lp[:], axis=AX.X)
                nc.sync.dma_start(out_loss[:], lsum[:])

    nc.compile()
    return nc


def _quant_w(W, kb):
    """W [K, 4096] f32 -> fp8 [128, kb*2*8*512] in (kb, dr, nc, n) layout,
    scaled by WS."""
    K = W.shape[0]
    assert K == kb * 256
    Wq = (W * WS).reshape(kb, 2, 128, 8, 512)       # [kb, dr, p, nc, n]
    Wq = Wq.transpose(2, 0, 1, 3, 4).reshape(128, kb * 2 * 8 * 512)
    return np.ascontiguousarray(Wq).astype(FP8NP)


def _prep_inputs(inputs):
    f32 = np.float32
    enc = np.asarray(inputs["encoded"], f32)
    est = np.asarray(inputs["encoder_state"], f32)
    tok = np.asarray(inputs["tgt_tokens"]).astype(np.int32)
    enc_lens = np.asarray(inputs["enc_lens"]).astype(np.int32)
    tgt_lens = np.asarray(inputs["tgt_lens"]).astype(np.int32)
    E = np.asarray(inputs["E"], f32)
    P = np.asarray(inputs["P"], f32)
    W0 = np.asarray(inputs["W0"], f32)
    U0 = np.asarray(inputs["U0"], f32)
    W1 = np.asarray(inputs["W1"], f32)
    U1 = np.asarray(inputs["U1"], f32)
    Wo = np.asarray(inputs["Wo"], f32)
    bo = np.asarray(inputs["bo"], f32)
    b0 = np.asarray(inputs["b0"], f32)
    b1 = np.asarray(inputs["b1"], f32)
    assert not b0.any() and not b1.any(), "nonzero LSTM bias unsupported"

    eT = np.ascontiguousarray(E.T).astype(BF16NP)
    pTb = np.ascontiguousarray(P.T).astype(BF16NP)
    wo_b = Wo.astype(BF16NP)
    wc_b = np.ascontiguousarray(W0[EMB:]).astype(BF16NP)
    boT = np.ascontiguousarray(bo.reshape(4, 128).T)          # [128, 4]

    # fold P into the embedding input weights; pad K 128 -> 256 with zeros
    w0e_f = np.zeros((256, 4 * H), f32)
    w0e_f[:RANK] = P @ W0[:EMB]
    w0e_q = _quant_w(w0e_f, 1)
    u0_q = _quant_w(U0, 4)
    w1_q = _quant_w(W1, 4)
    u1_q = _quant_w(U1, 4)

    def hq_init(h):   # h [BS, H] -> fp8 [128, (kb, dr, PB)] padded
        ht = h.T.reshape(KB, 2, 128, BS).transpose(2, 0, 1, 3)
        out = np.zeros((128, KB, 2, PB), np.float32)
        out[:, :, :, :BS] = ht
        return np.ascontiguousarray(out.reshape(128, KB * 2 * PB)).astype(FP8NP)

    in_maps = []
    for k in range(NCORES):
        ob = slice(k * BS, (k + 1) * BS)
        encz = enc[:, ob, :].copy()
        pad = np.arange(S)[:, None] >= enc_lens[ob][None, :]   # [S, BS]
        encz[pad, :] = 0.0
        encTz = encz.transpose(2, 1, 0)                        # [C, BS, S]
        # pair-padded layout [128, (ch, pair, 2*SPAD)]
        encTP_o = np.zeros((C, 2, 2 * SPAD), f32)
        for b in range(BS):
            encTP_o[:, b // 2, (b % 2) * SPAD:(b % 2) * SPAD + S] = \
                encTz[:, b, :]
        encTP_o = np.ascontiguousarray(
            encTP_o.reshape(NCH, 128, 2 * 2 * SPAD)
            .transpose(1, 0, 2).reshape(128, NCH * 2 * 2 * SPAD)).astype(BF16NP)
        sm = -(S - enc_lens[ob]).astype(f32).reshape(BS, 1)
        dm = np.zeros((BS, BS * S), f32)
        for j in range(BS):
            dm[j, j * S:(j + 1) * S] = 1.0
        tokid = np.ascontiguousarray(
            tok[ob, :TS].T.reshape(NROW, 1)).astype(np.int32)
        lab = np.ascontiguousarray(
            tok[ob, 1:T].T.reshape(NROW, 1)).astype(np.int32)
        vm = (np.arange(TS)[:, None] <
              (tgt_lens[ob] - 1)[None, :]).astype(f32).reshape(NROW)
        vmp = np.zeros((128, 2), f32)
        vmp[:128, 0] = vm[:128]
        vmp[:NROW - 128, 1] = vm[128:]
        in_maps.append({
            "w0e": w0e_q, "u0": u0_q, "w1": w1_q, "u1": u1_q,
            "wc": wc_b, "wo": wo_b, "boT": boT, "pTb": pTb,
            "encTP": encTP_o, "smaskT": sm, "dmask": dm,
            "h0q_i": hq_init(est[0, ob]), "h1q_i": hq_init(est[1, ob]),
            "e_rows": E, "eT": eT,
            "tokidx": tokid, "labidx": lab, "vmask": vmp,
        })
    return in_maps


_NC_CACHE = {}


def kernel(**inputs) -> np.ndarray:
    if "nc" not in _NC_CACHE:
        _NC_CACHE["nc"] = build()
    nc = _NC_CACHE["nc"]
    in_maps = _prep_inputs(inputs)
    res = bass_utils.run_bass_kernel_spmd(
        nc, in_maps, core_ids=list(range(NCORES)))
    _NC_CACHE["res"] = res
    total = sum(np.float32(res.results[c]["loss"][0, 0])
                for c in range(NCORES))
    return np.float32(total)
